# revision 17
# baseline (speedup 1.0000x reference)
"""MoE (top-2 of 8 experts + shared expert) Trainium2 kernel, expert-parallel
across 8 NeuronCores.

Strategy:
  - Host: compute the (tiny) gate in float64 numpy, select top-2 experts per
    token, and dispatch tokens by routing index (the all-to-all of
    expert-parallel MoE, done during the host-side shard step).
  - Work is balanced by slot packing: every core runs 3 routed fixed-capacity
    token slots plus one shared-expert slot of 512 tokens.  The slot caps are
    chosen at runtime by a small search that minimizes total capacity
    (8*sum(caps) >= routed tokens) subject to an exact-cover feasibility DP,
    so padding waste is ~1-2% instead of the 25% a fixed two-cap scheme costs.
  - Device (per core): feature-major MLP per slot, all operands bf16 (full
    PE rate, half the DMA traffic of fp32).  x^T and h stay resident in SBUF;
    weights stream it-tile by it-tile as one fused DMA per i-tile (w1e|w3e|
    w1o|w3o concatenated -> 8KB contiguous lines); swiglu is 5 DVE + 3 ACT
    ops per tile (Silu activation fuses sigmoid*mul); second GEMM accumulates
    over 16 i-tiles and writes bf16 y.
  - Scheduling: slots run largest-cap first (best PE-work-per-weight-byte
    while the DMA pipeline is cold); the next slot's x/bias loads trigger
    from the GpSimd queue and y writebacks from the Scalar queue so the
    in-order Sync queue only carries weight loads (no head-of-line blocking
    behind y-ready semaphores); 5 weight i-tiles of the next slot prefetch
    ahead of each slot's second GEMM.  Measured ~445-465us vs the 598us
    fp32r two-slot baseline (PE busy ~423us vs a ~414us matmul-row floor,
    MFU ~91%; run-to-run spread above that is device power throttling).
  - Host: combine = scatter-add of per-piece outputs weighted by the gate
    probabilities (1.0 for shared slices).  The swiglu even/odd interleave
    split, transposes, and the 1/1.702 silu rescale are pre-folded into the
    host-side weight layouts.
"""
import sys

sys.path.insert(0, "/opt/trn_rl_repo")

import itertools

import ml_dtypes
import numpy as np

import concourse.bacc as bacc_mod
import concourse.tile as tile
from concourse import mybir
from concourse.bass_utils import run_bass_kernel_spmd

F32 = mybir.dt.float32
BF16 = mybir.dt.bfloat16
NP_BF16 = ml_dtypes.bfloat16
Alu = mybir.AluOpType
Act = mybir.ActivationFunctionType

ALPHA = 1.702
LIMIT = 7.0
TOPK = 2
D, I, E = 1024, 2048, 8
B, S = 2, 2048
T = B * S
DK = D // 128          # 8 d-tiles
IT = I // 128          # 16 i-tiles
TS = 512               # shared-expert tokens per core (T / 8)
N_CORES = 8
NB = 4 * IT + DK       # bias-pack columns

_kernel_cache = {}


# --------------------------------------------------------------------------
# slot-cap planning: minimize total per-core routed capacity subject to an
# exact-cover feasibility DP (each cap has 8 instances, one per core).
# --------------------------------------------------------------------------

def _cover(caps, counts, ninst=8):
    """Assign instances of each cap to experts covering counts.
    Returns per-expert tuples n_ej or None if infeasible."""
    k = len(caps)
    per = []
    for cnt in counts:
        out = []
        for combo in itertools.product(range(ninst + 1), repeat=k):
            cap = sum(n * c for n, c in zip(combo, caps))
            if cap >= cnt:
                out.append((cap - cnt, combo))
        if not out:
            return None
        out.sort()
        per.append([c for _, c in out[:64]])
    memo = {}

    def dp(i, used):
        if i == len(counts):
            return []
        key = (i, used)
        if key in memo:
            return memo[key]
        res = None
        for combo in per[i]:
            nu = tuple(u + n for u, n in zip(used, combo))
            if any(u > ninst for u in nu):
                continue
            sub = dp(i + 1, nu)
            if sub is not None:
                res = [combo] + sub
                break
        memo[key] = res
        return res

    return dp(0, (0,) * k)


def _caps_at(C, counts, step):
    """All feasible (caps, asg) at capacity C, preferring a large min cap."""
    best = None
    for c1 in range(min(512, C - 384), 191, -step):
        for c2 in range(min(c1, C - c1 - 192), 191, -step):
            c3 = C - c1 - c2
            if c3 < 192 or c3 > c2:
                continue
            if best is not None and c3 <= best[0][2]:
                continue
            asg = _cover([c1, c2, c3], counts)
            if asg is not None:
                best = ([c1, c2, c3], asg)
    return best


def _plan_caps(counts):
    """Pick routed slot caps (each in [192, 512]) minimizing per-core
    capacity, then preferring balanced caps. Returns (caps, assignment)."""
    for C in range(1024, 1296, 16):
        best = _caps_at(C, counts, 16)
        if best is not None:
            # step-8 refinement: try to shave one 8-token notch
            fine = _caps_at(C - 8, counts, 8)
            return fine if fine is not None else best
    # robust fallback: always feasible (capacity 12288 >= 8192, and any
    # expert count <= 4096 = 8*512 spreads over equal cap positions)
    caps = [512, 512, 512]
    asg = _cover(caps, counts)
    if asg is None:
        caps = [512, 512, 512, 512]
        asg = _cover(caps, counts)
    return caps, asg


# --------------------------------------------------------------------------
# host-side packing to device layouts (all bf16 except biases)
# --------------------------------------------------------------------------

def _tile13(w):
    """[D, I] -> [IT, 128(k), DK, 128(m)]  (k = d%128, m = i%128)."""
    return w.reshape(DK, 128, IT, 128).transpose(2, 1, 0, 3)


def _expert_pack(w1, b1, w3, b3, w2, b2):
    wf = np.stack([_tile13(w1[:, 0::2]), _tile13(w3[:, 0::2]),
                   _tile13(w1[:, 1::2]), _tile13(w3[:, 1::2])], axis=2)
    wf = np.ascontiguousarray(wf.reshape(IT, 128, 4 * DK * 128))
    w2t = (w2 * np.float32(1.0 / ALPHA)).reshape(IT, 128, DK, 128)
    w2t = np.ascontiguousarray(w2t.transpose(2, 1, 0, 3).reshape(DK, 128, IT * 128))
    bias = np.concatenate([
        b1[0::2].reshape(IT, 128).T, b3[0::2].reshape(IT, 128).T,
        b1[1::2].reshape(IT, 128).T, b3[1::2].reshape(IT, 128).T,
        b2.reshape(DK, 128).T,
    ], axis=1)
    return {
        "wf": wf.astype(NP_BF16),
        "w2": w2t.astype(NP_BF16),
        "bias": np.ascontiguousarray(bias, dtype=np.float32),
    }


def _xt_pack(xsub, cap):
    """[n, D] tokens -> zero-padded [128, DK*cap] bf16 transposed layout."""
    n = xsub.shape[0]
    xt = np.zeros((D, cap), dtype=np.float32)
    xt[:, :n] = xsub.T
    xt = xt.reshape(DK, 128, cap).transpose(1, 0, 2)
    return np.ascontiguousarray(xt.reshape(128, DK * cap)).astype(NP_BF16)


# --------------------------------------------------------------------------
# device kernel
# --------------------------------------------------------------------------

def _groups(cap):
    gs = [512] * (cap // 512)
    if cap % 512:
        gs.append(cap % 512)
    offs = np.cumsum([0] + gs)[:-1]
    return list(zip(offs, gs))


def _build(caps):
    """Build the SPMD Bass kernel; caps = routed slot caps + [TS] shared."""
    nc = bacc_mod.Bacc("TRN2")

    def dram(name, shape, dtype, out=False):
        return nc.declare_dram_parameter(name, list(shape), dtype, isOutput=out)

    slots = []
    for j, cap in enumerate(caps):
        p = f"s{j}"
        w = {
            "xt": dram(p + "xt", [128, DK * cap], BF16),
            "wf": dram(p + "wf", [IT, 128, 4 * DK * 128], BF16),
            "w2": dram(p + "w2", [DK, 128, IT * 128], BF16),
            "bias": dram(p + "bias", [128, NB], F32),
            "y": dram(p + "y", [DK, 128, cap], BF16, out=True),
        }
        slots.append((j, cap, w))

    with tile.TileContext(nc) as tc:
        with (
            tc.tile_pool(name="persist", bufs=1) as persist,
            tc.tile_pool(name="wpool", bufs=6) as wpool,
            tc.tile_pool(name="w2pool", bufs=4) as w2pool,
            tc.tile_pool(name="work", bufs=2) as work,
            tc.tile_pool(name="outp", bufs=3) as outp,
            tc.tile_pool(name="ps", bufs=1, space="PSUM") as ps,
            tc.tile_pool(name="psy", bufs=3, space="PSUM") as psy,
        ):
            # persistent per-slot tiles, allocated upfront

            xts_t, bt_t, hb_t = {}, {}, {}
            for j, cap, w in slots:
                xts_t[j] = persist.tile([128, DK * cap], BF16, tag=f"xt{j}",
                                        name=f"xt_s{j}")
                bt_t[j] = persist.tile([128, NB], F32, tag=f"bias{j}",
                                       name=f"bias_s{j}")
                hb_t[j] = persist.tile([128, IT * cap], BF16, tag=f"h{j}",
                                       name=f"h_s{j}")

            def load_xt_bias(j):
                # gpsimd-queue triggers: keeps these off the Sync queue so
                # weight-load triggers are never stuck behind them
                _, cap, w = slots[j]
                if j == 0:
                    # split halves so the first matmuls wait on less data
                    half = DK * cap // 2
                    xap = w["xt"].ap()
                    nc.gpsimd.dma_start(out=xts_t[j][:, :half],
                                        in_=xap[:, :half])
                    nc.gpsimd.dma_start(out=xts_t[j][:, half:],
                                        in_=xap[:, half:])
                else:
                    nc.gpsimd.dma_start(out=xts_t[j], in_=w["xt"].ap())
                nc.gpsimd.dma_start(out=bt_t[j], in_=w["bias"].ap())

            load_xt_bias(0)

            def wf_load(j, it):
                _, _, w = slots[j]
                wt = wpool.tile([128, 4 * DK * 128], BF16, tag="wf",
                                name=f"wf_s{j}_{it}")
                if j == 0 and it == 0:
                    # split per-w so the first matmul waits on 1/4 of it
                    for wi in range(4):
                        o = wi * DK * 128
                        nc.sync.dma_start(out=wt[:, o:o + DK * 128],
                                          in_=w["wf"][it][:, o:o + DK * 128])
                else:
                    nc.sync.dma_start(out=wt, in_=w["wf"][it])
                return wt

            wf_pre = {}
            for j, cap, w in slots:
                grp = _groups(cap)
                xts, bt, hb = xts_t[j], bt_t[j], hb_t[j]
                w2_pre = {}

                def prefetch_w2(dk):
                    w2t = w2pool.tile([128, IT * 128], BF16, tag="w2",
                                      name=f"w2_s{j}_{dk}")
                    nc.sync.dma_start(out=w2t, in_=w["w2"][dk])
                    w2_pre[dk] = w2t

                # ---- first GEMM + swiglu: h[it, tok] ----
                for it in range(IT):
                    wt = wf_pre.pop((j, it), None)
                    if wt is None:
                        wt = wf_load(j, it)
                    if it == 8 and j + 1 < len(slots):
                        load_xt_bias(j + 1)          # prefetch next slot x
                    if it in (9, 11, 13, 15):
                        prefetch_w2((it - 9) // 2)   # prefetch w2 head
                    for goff, gsz in grp:
                        accs = []
                        for wi in range(4):
                            acc = ps.tile([128, 512], F32, tag=f"acc{wi}",
                                          name=f"acc{wi}_s{j}_{it}_{goff}")
                            for dk in range(DK):
                                o = (wi * DK + dk) * 128
                                nc.tensor.matmul(
                                    acc[:, :gsz],
                                    wt[:, o:o + 128],
                                    xts[:, dk * cap + goff:
                                        dk * cap + goff + gsz],
                                    start=(dk == 0), stop=(dk == DK - 1))
                            accs.append(acc)
                        A, Bm, C, Dm = accs
                        Bp = work.tile([128, 512], F32, tag="Bp")
                        nc.scalar.activation(Bp[:, :gsz], Bm[:, :gsz],
                                             Act.Identity,
                                             bias=bt[:, IT + it:IT + it + 1])
                        G = work.tile([128, 512], F32, tag="G")
                        nc.vector.scalar_tensor_tensor(
                            G[:, :gsz], A[:, :gsz], bt[:, it:it + 1],
                            Bp[:, :gsz], Alu.add, Alu.mult)
                        nc.vector.tensor_scalar_min(G[:, :gsz], G[:, :gsz],
                                                    LIMIT)
                        # Sv = silu(alpha*G); the 1/alpha rescale is folded
                        # into w2 on the host
                        Sv = work.tile([128, 512], F32, tag="Sv")
                        nc.scalar.activation(Sv[:, :gsz], G[:, :gsz],
                                             Act.Silu, scale=ALPHA)
                        Dp = work.tile([128, 512], F32, tag="Dp")
                        nc.scalar.activation(
                            Dp[:, :gsz], Dm[:, :gsz], Act.Identity,
                            bias=bt[:, 3 * IT + it:3 * IT + it + 1])
                        L = work.tile([128, 512], F32, tag="L")
                        nc.vector.scalar_tensor_tensor(
                            L[:, :gsz], C[:, :gsz],
                            bt[:, 2 * IT + it:2 * IT + it + 1],
                            Dp[:, :gsz], Alu.add, Alu.mult)
                        nc.vector.tensor_scalar(L[:, :gsz], L[:, :gsz],
                                                LIMIT, -LIMIT,
                                                Alu.min, Alu.max)
                        nc.vector.scalar_tensor_tensor(
                            hb[:, it * cap + goff:it * cap + goff + gsz],
                            L[:, :gsz], 1.0, Sv[:, :gsz], Alu.add, Alu.mult)

                # prefetch the next slot's first weight tiles ahead of the
                # GEMM2 y-writeback triggers (Sync queue is in-order)
                if j + 1 < len(slots):
                    for it2 in range(5):
                        wf_pre[(j + 1, it2)] = wf_load(j + 1, it2)

                # ---- second GEMM: y[dk] = sum_it w2[dk,it].T @ h[it] ----
                for dk in range(DK):
                    if dk in w2_pre:
                        w2t = w2_pre.pop(dk)
                    else:
                        w2t = w2pool.tile([128, IT * 128], BF16, tag="w2",
                                          name=f"w2_s{j}_{dk}")
                        nc.sync.dma_start(out=w2t, in_=w["w2"][dk])
                    for goff, gsz in grp:
                        Y = psy.tile([128, 512], F32, tag="Y",
                                     name=f"Y_s{j}_{dk}_{goff}")
                        for it in range(IT):
                            nc.tensor.matmul(
                                Y[:, :gsz],
                                w2t[:, it * 128:(it + 1) * 128],
                                hb[:, it * cap + goff:it * cap + goff + gsz],
                                start=(it == 0), stop=(it == IT - 1))
                        yo = outp.tile([128, 512], BF16, tag="yo")
                        nc.scalar.activation(
                            yo[:, :gsz], Y[:, :gsz], Act.Identity,
                            bias=bt[:, 4 * IT + dk:4 * IT + dk + 1])
                        # scalar-queue trigger: fires right after the ACT
                        # above with no semaphore wait, and keeps y
                        # writebacks from head-of-line blocking Sync
                        nc.scalar.dma_start(
                            out=w["y"][dk, :, goff:goff + gsz],
                            in_=yo[:, :gsz])

    nc.finalize()
    return nc


# --------------------------------------------------------------------------
# entry point
# --------------------------------------------------------------------------

def kernel(x, gate_w, gate_b, w1, b1, w3, b3, w2, b2,
           sw1, sb1, sw3, sb3, sw2, sb2):
    x = np.asarray(x, dtype=np.float32)
    xt = x.reshape(T, D)

    # ---- gate (float64 host math; selection + combine weights) ----
    z = xt.astype(np.float64) @ np.asarray(gate_w, dtype=np.float64).T
    z -= z.max(axis=-1, keepdims=True)
    ez = np.exp(z)
    scores = ez / ez.sum(axis=-1, keepdims=True)          # [T, E]
    biased = scores + np.asarray(gate_b, dtype=np.float64)
    top2 = np.argsort(-biased, axis=-1, kind="stable")[:, :TOPK]   # [T, 2]
    gate_wt = np.take_along_axis(scores, top2, axis=-1).astype(np.float32)

    tok_idx = []
    tok_wt = []
    for e in range(E):
        sel = np.nonzero((top2 == e).any(axis=1))[0]
        we = np.where(top2[sel, 0] == e, gate_wt[sel, 0], gate_wt[sel, 1])
        tok_idx.append(sel)
        tok_wt.append(we.astype(np.float32))
    counts = [len(s) for s in tok_idx]

    # ---- plan slot caps + cut expert token lists into per-slot pieces ----
    rcaps, assign = _plan_caps(counts)
    k = len(rcaps)
    pieces = {j: [] for j in range(k)}       # slot idx -> list of (e, lo, hi)
    for e in range(E):
        lo = 0
        for j in range(k):
            for _ in range(assign[e][j]):
                hi = min(lo + rcaps[j], counts[e])
                pieces[j].append((e, lo, hi))
                lo = hi
        assert lo >= counts[e]
    for j in range(k):
        while len(pieces[j]) < N_CORES:
            pieces[j].append((0, 0, 0))

    # ---- build per-core input maps ----
    epacks = [
        _expert_pack(np.asarray(w1[e]), np.asarray(b1[e]),
                     np.asarray(w3[e]), np.asarray(b3[e]),
                     np.asarray(w2[e]), np.asarray(b2[e]))
        for e in range(E)
    ]
    spack = _expert_pack(np.asarray(sw1), np.asarray(sb1),
                         np.asarray(sw3), np.asarray(sb3),
                         np.asarray(sw2), np.asarray(sb2))
    slot_kinds = sorted([(rcaps[j], j) for j in range(k)] + [(TS, -1)],
                        key=lambda t: -t[0])        # largest cap first
    caps = tuple(cap for cap, _ in slot_kinds)
    in_maps = []
    for c in range(N_CORES):
        m = {}
        for s, (cap, kidx) in enumerate(slot_kinds):
            if kidx < 0:
                m[f"s{s}xt"] = _xt_pack(xt[c * TS:(c + 1) * TS], TS)
                pk = spack
            else:
                e, lo, hi = pieces[kidx][c]
                m[f"s{s}xt"] = _xt_pack(xt[tok_idx[e][lo:hi]], cap)
                pk = epacks[e]
            for kk, v in pk.items():
                m[f"s{s}{kk}"] = v
        in_maps.append(m)

    # ---- compile (cached) + run on all 8 cores ----
    if caps not in _kernel_cache:
        _kernel_cache[caps] = _build(caps)
    nc = _kernel_cache[caps]
    res = run_bass_kernel_spmd(nc, in_maps, list(range(N_CORES)))

    # ---- combine: weighted scatter-add of routed pieces + shared slices ----
    out = np.zeros((T, D), dtype=np.float32)
    for c in range(N_CORES):
        for s, (cap, kidx) in enumerate(slot_kinds):
            yc = res.results[c][f"s{s}y"].astype(np.float32).reshape(D, cap)
            if kidx < 0:
                out[c * TS:(c + 1) * TS] += yc.T
            else:
                e, lo, hi = pieces[kidx][c]
                if hi <= lo:
                    continue
                idx = tok_idx[e][lo:hi]
                out[idx] += tok_wt[e][lo:hi][:, None] * yc.T[:hi - lo]
    return out.reshape(B, S, D)
